# revision 49
# baseline (speedup 1.0000x reference)
"""Trainium2 Bass kernel for nn_ConvexMLPBlock.

Reference computation (B=64, HW=196, D=768, E=256, C=10):
    S[b,h,e]  = (x[b,h,:] @ ag_w[e,:] + ag_b[e]) > 0          (sign patterns)
    z[b,h,p]  = x[b,h,:] @ lm_w[p,:]        (p = e*C + c)
    preds[b,c] = sum_{h,e} S[b,h,e] * z[b,h,e,c] / (HW*E)

Restructured to avoid materializing z (49 GFLOP -> ~10 GFLOP):
    G_b[e,d]   = sum_h S[b,h,e] * x[b,h,d]                    (per-batch masked moment)
    preds[b,c] = (1/(HW*E)) * sum_{e,d} G_b[e,d] * W[e,c,d]   (W = lm_w.reshape(E,C,D))

Sharding: data-parallel over B across the 8 NeuronCores (8 batches/core);
host concatenates the per-core (8,10) outputs.

Per-core pipeline (final, ~48us HW vs the 104us fp16x3+transpose baseline):
    warm-up: ~23 dummy matmuls cover the DMA ramp so the PE HAM clock-gate
         releases (1.2 -> 2.4 GHz) before real work starts.
    mm1: S[t,e] directly (stationary = x^T d-chunks, moving = ag^T [d,256]),
         ONE fp16 pass (rel err ~1.0e-2 < 2e-2 gate; fp16 products are exact
         in the PE, error comes only from operand rounding). No transposes.
    threshold: DVE tensor_tensor is_gt vs a broadcast (-ag_b) tile.
    mm2: G^T_b[d,e] contraction over h (stationary = x natural d-slices,
         moving = S), 2 h-tiles per batch, fp16. Interleaved with mm1 per
         batch pair so the PE stream is ~2x denser than the x DMA stream.
    final: the e-diagonal selection mask is group-independent, so ALL 96
           cross-product matmuls (per d-tile and e-group: stationary
           G^T[d,(b,e)], moving W[d,(c,e)]) accumulate into a single
           [128,(c,e)] PSUM tile; then one mask-mult with the 1/(HW*E)
           scale folded in (DVE scalar_tensor_tensor), one sel3
           partition-sum matmul, one e-reduce.
    DMA plan (exec time tracks last-DMA-completion): big tensors ride the
    two HWDGE rings in consumption order -- SP: agt, then xt/xn batch
    pairs interleaved, then wfinA; ACT: xt01 (concurrent with agt), negb,
    small tensors, gt copies. SWDGE (gpsimd) carries only wfinB behind an
    mm1-progress stagger (sem-wait blocking is harmless there) and the
    tiny output. Each DMA issue costs ~0.65us serialized per ring and
    ~2us completion latency, so few, large, contiguous-per-partition
    descriptors win; deps that block a HWDGE ring queue stall every DMA
    behind them (the v6 negb lesson).
"""

import numpy as np

import concourse.bass as bass
import concourse.mybir as mybir
import concourse.tile as tile
from concourse.tile import add_dep_helper
from concourse.bass_utils import run_bass_kernel_spmd

# Problem constants (hardcoded per contract).
B = 64
HW = 196
D = 768
E = 256
C = 10
NCORES = 8
BL = B // NCORES          # local batches per core = 8
T = BL * HW               # local tokens = 1568
KT = D // 128             # 6 d-tiles
EG = 16                   # e's per final-stage group
NG = E // EG              # 16 groups

FP32 = mybir.dt.float32
BF16 = mybir.dt.bfloat16
FP16 = mybir.dt.float16


def _patched_drain_and_barrier(self, tick_clock, wait_clock):
    """This toolchain's walrus rejects >1 sync-wait on CTRL-class (Drain)
    instructions. Split the tail drain's global-clock waits across multiple
    single-wait drains. Semantics preserved: SP observes every DMA-queue
    semaphore before the all-engine barrier."""
    drain_inst = self.nc.sync.drain()
    wait_clock.add_sem_waits(
        drain_inst.ins, tile.ScopedClock({None: tick_clock.global_clock})
    )
    si = drain_inst.ins.sync_info
    if si is not None and si.on_wait is not None and len(si.on_wait) > 1:
        waits = list(si.on_wait)
        drain_inst.ins.sync_info = mybir.SyncInfo(
            on_wait=[waits[0]], on_update=list(si.on_update or [])
        )
        for w in waits[1:]:
            extra = self.nc.sync.drain()
            extra.ins.sync_info = mybir.SyncInfo(on_wait=[w], on_update=[])

    self.nc.all_engine_barrier()
    assert self.sems is not None
    popped = self.nc._tile_sem_poison_stack.pop()
    assert popped is self._sem_poison
    self.nc.clear_and_free_semaphores(list(self.sems.allocated().values()))
    self.nc.all_engine_barrier()


tile.TileContext._drain_and_barrier = _patched_drain_and_barrier


def _split_multiwait_json(bj: bytes) -> bytes:
    """Walrus in this toolchain accepts at most one sync-wait per instruction.
    For any instruction with N>1 waits, hoist N-1 waits onto same-engine NoOps
    inserted immediately before it. Engines execute program-order, so for
    compute instructions this is semantically identical; for DMAs it
    conservatively blocks the issuing engine instead of the queue."""
    import json

    m = json.loads(bj)
    changed = False
    for fn in m["functions"]:
        for bb in fn["blocks"]:
            new_insts = []
            for inst in bb["instructions"]:
                si = inst.get("sync_info")
                ow = (si or {}).get("on_wait") or []
                if len(ow) > 1:
                    for j, w in enumerate(ow[:-1]):
                        new_insts.append(
                            {
                                "name": f"{inst['name']}__w{j}",
                                "opcode": "NoOp",
                                "engine": inst["engine"],
                                "ins": [],
                                "outs": [],
                                "sync_info": {"on_update": [], "on_wait": [w]},
                            }
                        )
                    si["on_wait"] = [ow[-1]]
                    changed = True
                new_insts.append(inst)
            bb["instructions"] = new_insts
    if not changed:
        return bj
    return json.dumps(m).encode()


_orig_to_json_bytes = bass.Bass.to_json_bytes


def _patched_to_json_bytes(self, *a, **k):
    return _split_multiwait_json(_orig_to_json_bytes(self, *a, **k))


bass.Bass.to_json_bytes = _patched_to_json_bytes


# (batch, half) chunks: per batch a 128-row and a 68-row h-chunk.
CHUNKS = []
for _b in range(BL):
    CHUNKS.append((_b, 0, 0, 128))
    CHUNKS.append((_b, 1, 128, HW - 128))


def build_program(phases=("mm1", "mm2", "fin")):
    nc = bass.Bass()

    # xt[dp, b, kt, h] = x_core[b*HW+h, kt*128+dp]   (fp16, mm1 stationary)
    xt_d = nc.dram_tensor("xt", (128, BL, KT, HW), FP16,
                          kind="ExternalInput").ap()
    # agt[dp, kt, e] = ag_w[e, kt*128+dp]            (fp16, mm1 moving)
    agt_d = nc.dram_tensor("agt", (128, KT, E), FP16, kind="ExternalInput").ap()
    # negb[p, e] = -ag_b[e]                          (fp32, threshold)
    negb_d = nc.dram_tensor("negb", (128, E), FP32, kind="ExternalInput").ap()
    # xn_pk[p, ch, d] = x_core[chunk ch row p, d] (chunk-packed, tail-padded)
    xn_d = nc.dram_tensor("xn", (128, 2 * BL, D), FP16,
                          kind="ExternalInput").ap()
    # wfin[dp, gh, kt, gl, c, el] = lm_w[((gh*8+gl)*EG+el)*C+c, kt*128+dp]
    wfin_d = nc.dram_tensor("wfin", (128, 2, KT, NG // 2, C, EG), FP16,
                            kind="ExternalInput").ap()
    # mask[b*EG+ep, c, el] = (ep == el)
    mask_d = nc.dram_tensor("mask", (128, C, EG), FP16,
                            kind="ExternalInput").ap()
    # sel3[b*EG+ep, bp] = (b == bp)
    sel3_d = nc.dram_tensor("sel3", (128, BL), FP16, kind="ExternalInput").ap()
    preds_o = nc.dram_tensor("preds_o", (BL, C), FP32, kind="ExternalOutput").ap()

    from contextlib import ExitStack
    with tile.TileContext(nc) as tc, ExitStack() as _es:
        xt_p = _es.enter_context(tc.tile_pool(name="xt_p", bufs=1))
        agt_p = _es.enter_context(tc.tile_pool(name="agt_p", bufs=1))
        small_p = _es.enter_context(tc.tile_pool(name="small_p", bufs=1))
        sn_p = _es.enter_context(tc.tile_pool(name="sn_p", bufs=1))
        xn_p = _es.enter_context(tc.tile_pool(name="xn_p", bufs=1))
        gt_p = _es.enter_context(tc.tile_pool(name="gt_p", bufs=1))
        wfin_p = _es.enter_context(tc.tile_pool(name="wfin_p", bufs=1))
        out_p = _es.enter_context(tc.tile_pool(name="out_p", bufs=1))
        ps1 = _es.enter_context(tc.tile_pool(name="ps1", bufs=4, space="PSUM"))
        ps2 = _es.enter_context(tc.tile_pool(name="ps2", bufs=3, space="PSUM"))
        psM = _es.enter_context(tc.tile_pool(name="psM", bufs=1, space="PSUM"))

        # ---- PE warm-up: HAM releases the PE clock gate (1.2 -> 2.4 GHz)
        # only after ~3.4us of sustained matmul activity; the first few us
        # are DMA-bound. Memsets ride DVE so the warm matmuls start at ~0.
        warm_src = small_p.tile([128, E], FP16, tag="warm_src",
                                name="warm_src")
        nc.vector.memset(warm_src[:], 0.0)
        warm_w = small_p.tile([128, 128], FP16, tag="warm_w", name="warm_w")
        nc.vector.memset(warm_w[:], 0.0)
        for wi in range(20):
            wps = ps1.tile([128, E], FP32, tag="ps1", name=f"warm_ps{wi}")
            nc.tensor.matmul(
                wps[:], warm_w[:], warm_src[:], start=True, stop=True
            )

        # ---- persistent loads, consumption order, issue ~0.65us each
        # serialized per ring. The first two tensors mm1 needs (agt, xt01)
        # ride DIFFERENT rings so they transfer concurrently; the rest of
        # the xt/xn stream alternates on SP. ACT ring: small tensors (it
        # also runs gt copies). SWDGE (gpsimd): wfinB behind a stagger.
        agt_sb = agt_p.tile([128, KT, E], FP16, tag="agt", name="agt_sb")
        nc.sync.dma_start(agt_sb[:], agt_d[:, :, :])
        xt_sb = xt_p.tile([128, BL, KT, HW], FP16, tag="xt", name="xt_sb")
        xn_sb = xn_p.tile([128, 2 * BL, D], FP16, tag="xn", name="xn_sb")
        nc.scalar.dma_start(xt_sb[:, 0:2, :, :], xt_d[:, 0:2, :, :])
        nc.sync.dma_start(xn_sb[:, 0:4, :], xn_d[:, 0:4, :])
        for bp in range(1, 4):
            nc.sync.dma_start(xt_sb[:, 2 * bp:2 * bp + 2, :, :],
                              xt_d[:, 2 * bp:2 * bp + 2, :, :])
            nc.sync.dma_start(xn_sb[:, 4 * bp:4 * bp + 4, :],
                              xn_d[:, 4 * bp:4 * bp + 4, :])

        negb_sb = small_p.tile([128, E], FP32, tag="negb", name="negb_sb")
        nc.scalar.dma_start(negb_sb[:], negb_d[:, :])
        mask_sb = small_p.tile([128, C, EG], FP16, tag="mask", name="mask_sb")
        nc.scalar.dma_start(mask_sb[:], mask_d[:, :, :])
        sel3_sb = small_p.tile([128, BL], FP16, tag="sel3", name="sel3_sb")
        nc.scalar.dma_start(sel3_sb[:], sel3_d[:, :])
        # Pre-load the ACT op table (~1.3us, one-time) during the DMA phase
        # so the first real nc.scalar.copy doesn't stall the gt pipeline.
        act_warm = small_p.tile([128, 8], FP16, tag="act_warm",
                                name="act_warm")
        nc.scalar.copy(act_warm[:], warm_w[:, 0:8])

        # wfinA rides the SP ring right after the xt/xn stream (no dep --
        # it transfers while mm1/mm2 compute).
        wfin_sb = wfin_p.tile([128, 2, KT, NG // 2, C, EG], FP16, tag="wfin",
                              name="wfin_sb")
        nc.sync.dma_start(wfin_sb[:, 0, :, 0:4, :, :],
                          wfin_d[:, 0, :, 0:4, :, :])
        nc.sync.dma_start(wfin_sb[:, 0, :, 4:8, :, :],
                          wfin_d[:, 0, :, 4:8, :, :])

        # ---- mm1 + mm2, interleaved per batch pair so the PE stream is
        # ~2x denser than the xt/xn DMA stream (PE-bound, HAM stays warm).
        # mm1: S[t,e] = (x @ ag_w^T > -b), single fp16 pass.
        # mm2: G^T_b[d, e] = sum_h x[h,d] S[h,e];
        #      gt[dt][dp, g, b, el] = G^T_b[dt*128+dp, g*EG+el]
        sn_sb = [
            sn_p.tile([128, E], FP16, tag=f"sn{ch}", name=f"sn_sb{ch}")
            for ch in range(len(CHUNKS))
        ]
        gt_sb = [
            gt_p.tile([128, NG, BL, EG], FP16, tag=f"gt{dt}",
                      name=f"gt_sb{dt}")
            for dt in range(KT)
        ]
        th_insts = {}

        def emit_mm1(b):
            for ht in range(2):
                ch = 2 * b + ht
                _, _, h0, w = CHUNKS[ch]
                ps = ps1.tile([128, E], FP32, tag="ps1", name=f"ps1_{ch}")
                for kt in range(KT):
                    nc.tensor.matmul(
                        ps[0:w, :],
                        xt_sb[:, b, kt, h0:h0 + w],
                        agt_sb[:, kt, :],
                        start=(kt == 0),
                        stop=(kt == KT - 1),
                    )
                th_insts[ch] = nc.vector.tensor_tensor(
                    sn_sb[ch][0:w, :], ps[0:w, :], negb_sb[0:w, :],
                    mybir.AluOpType.is_gt,
                )

        def emit_mm2(b):
            for dt in range(KT):
                pg = ps2.tile([128, E], FP32, tag="ps2", name=f"ps2_{b}_{dt}")
                for ht in range(2):
                    ch = 2 * b + ht
                    w = CHUNKS[ch][3]
                    nc.tensor.matmul(
                        pg[:],
                        xn_sb[0:w, ch, dt * 128:(dt + 1) * 128],
                        sn_sb[ch][0:w, :],
                        start=(ht == 0),
                        stop=(ht == 1),
                    )
                if (b + dt) % 2 == 0:
                    nc.vector.tensor_copy(gt_sb[dt][:, :, b, :], pg[:])
                else:
                    nc.scalar.copy(gt_sb[dt][:, :, b, :], pg[:])

        for bp in range(4):
            emit_mm1(2 * bp)
            emit_mm1(2 * bp + 1)
            emit_mm2(2 * bp)
            emit_mm2(2 * bp + 1)
            if bp == 2:
                # wfinB on SWDGE once mm1 is well underway (late enough not
                # to steal HBM bandwidth from the xt/xn stream); blocking
                # the (otherwise idle) gpsimd queue on this stagger is
                # harmless.
                dma = nc.gpsimd.dma_start(wfin_sb[:, 1, :, :, :, :],
                                          wfin_d[:, 1, :, :, :, :])
                add_dep_helper(dma.ins, th_insts[9].ins,
                               reason="wfinB load after mm1 underway")

        # ---- final ----
        # mask (e-diagonal selection) is identical for every group, so it
        # commutes with the group sum: ALL cross-product matmuls accumulate
        # into one PSUM tile, masked once at the end.
        do_fin = "fin" in phases
        pm = psM.tile([128, C, EG], FP32, tag="psM", name="psM_t")
        if not do_fin:
            nc.vector.memset(pm[:], 0.0)
        nmm = KT * NG
        im = 0
        # group order matches wfin DMA arrival: wfinA-q1 (g0-3), wfinB
        # (g8-15, lands early on SWDGE), wfinA-q2 (g4-7, lands last on the
        # SP ring tail); the single-psum accumulation is order-free.
        GORDER = [0, 1, 2, 3, 8, 9, 10, 11, 12, 13, 14, 15, 4, 5, 6, 7]
        for g in (GORDER if do_fin else []):
            gh, gl = g // (NG // 2), g % (NG // 2)
            for dt in range(KT):
                nc.tensor.matmul(
                    pm[:],
                    gt_sb[dt][:, g, :, :],
                    wfin_sb[:, gh, dt, gl, :, :],
                    start=(im == 0),
                    stop=(im == nmm - 1),
                )
                im += 1
        # mask multiply with the 1/(HW*E) scale folded in, then the e-reduce
        # BEFORE the sel3 matmul: both DVE ops run back-to-back on one
        # queue, and sel3 shrinks from 160 to 10 moving columns.
        msb = out_p.tile([128, C, EG], FP32, tag="msb", name="msb_t")
        nc.vector.scalar_tensor_tensor(
            msb[:], pm[:], 1.0 / (HW * E), mask_sb[:],
            mybir.AluOpType.mult, mybir.AluOpType.mult,
        )
        msr = out_p.tile([128, C], FP16, tag="msr", name="msr_t")
        with nc.allow_low_precision(
            reason="16-term masked e-reduce to fp16; ~1e-4 rel on preds"
        ):
            nc.vector.tensor_reduce(
                msr[:], msb[:], mybir.AxisListType.X, mybir.AluOpType.add
            )
        # pf shares the ps2 pool (mm2 is long done); saves a PSUM bank
        # that ps1's 4th buffer uses instead
        pf = ps2.tile([128, E], FP32, tag="ps2", name="psf_t")
        nc.tensor.matmul(pf[0:BL, 0:C], sel3_sb[:], msr[:],
                         start=True, stop=True)
        out_sb = out_p.tile([BL, C], FP32, tag="out", name="out_sb")
        nc.vector.tensor_copy(out_sb[:], pf[0:BL, 0:C])
        # output rides SWDGE: the gpsimd queue is idle by now and its
        # descriptor generation for 8 rows is cheaper than a HWDGE
        # DIRECT2D issue (~0.78us)
        nc.gpsimd.dma_start(preds_o[:, :], out_sb[:])

    return nc


_program_cache = {}

CONFIG = {}


def _get_program(**kw):
    key = tuple(sorted(kw.items()))
    if key not in _program_cache:
        _program_cache[key] = build_program(**kw)
    return _program_cache[key]


def make_in_maps(x, ag_w, ag_b, lm_w, cfg=None):
    x = np.ascontiguousarray(np.asarray(x, dtype=np.float32))
    ag_w = np.asarray(ag_w, dtype=np.float32)
    ag_b = np.asarray(ag_b, dtype=np.float32)
    lm_w = np.asarray(lm_w, dtype=np.float32)

    agt = np.ascontiguousarray(
        ag_w.T.reshape(KT, 128, E).transpose(1, 0, 2).astype(np.float16)
    )
    negb = np.ascontiguousarray(
        np.broadcast_to(-ag_b[None, :], (128, E)).astype(np.float32)
    )
    # wfin[dp, gh, kt, gl, c, el] = lm_w[((gh*8+gl)*EG+el)*C+c, kt*128+dp]
    wfin = np.ascontiguousarray(
        lm_w.T.reshape(KT, 128, 2, NG // 2, EG, C)
        .transpose(1, 2, 0, 3, 5, 4)
        .astype(np.float16)
    )
    ep = np.arange(128) % EG
    mask = np.ascontiguousarray(
        (ep[:, None, None] == np.arange(EG)[None, None, :])
        * np.ones((128, C, EG), dtype=np.float16)
    )
    bidx = np.arange(128) // EG
    sel3 = (bidx[:, None] == np.arange(BL)[None, :]).astype(np.float16)

    common = {"agt": agt, "negb": negb, "wfin": wfin, "mask": mask,
              "sel3": sel3}
    in_maps = []
    for i in range(NCORES):
        xs = x[i * BL:(i + 1) * BL].reshape(T, D)
        m = dict(common)
        # xn_pk[p, (b,ht), d]: 128-row chunk + zero-padded 68-row tail chunk
        xn_pk = np.zeros((128, 2 * BL, D), dtype=np.float16)
        xsb = xs.reshape(BL, HW, D).astype(np.float16)
        for b in range(BL):
            xn_pk[:, 2 * b, :] = xsb[b, 0:128, :]
            xn_pk[0:HW - 128, 2 * b + 1, :] = xsb[b, 128:HW, :]
        m["xn"] = np.ascontiguousarray(xn_pk)
        # xt[dp, b, kt, h] = xs[b*HW+h, kt*128+dp]
        m["xt"] = np.ascontiguousarray(
            xs.T.reshape(KT, 128, BL, HW).transpose(1, 2, 0, 3)
            .astype(np.float16)
        )
        in_maps.append(m)
    return in_maps


def kernel(x, ag_w, ag_b, lm_w):
    in_maps = make_in_maps(x, ag_w, ag_b, lm_w)
    nc = _get_program()
    res = run_bass_kernel_spmd(nc, in_maps, core_ids=list(range(NCORES)))
    preds = np.concatenate(
        [res.results[i]["preds_o"] for i in range(NCORES)], axis=0
    )
    return np.ascontiguousarray(preds.astype(np.float32))
